# revision 24
# baseline (speedup 1.0000x reference)
"""Trainium2 Bass kernel for nn_EntityRoleClassifier (segment_reduce).

Strategy (data-parallel over batch, 8 NeuronCores):
  - Each core gets B/8 = 8 batch rows of sequence_output plus replicated MLP
    weights.
  - Per-entity left/span/right mean-pools are a dense matmul against
    host-built normalized interval masks (mask stationary, sequence moving),
    giving emb^T = mask^T @ seq in PSUM; DVE 32x32 block transposes land
    X^T (features on partitions) for the MLP.
  - The 4 head MLPs are fused: W1cat = [mW1|pW1|aW1|iW1] (2304x1024),
    W2blk = block-diagonal (1024x25).  The main head runs fp32 end-to-end
    (argmax near-ties need it); the p/a/i heads' W1 is bf16 (their sigmoids
    tolerate ~1e-4 error and it cuts weight DMA).
  - Softmax / argmax-select / sigmoid epilogue runs on-device, entity on
    partitions; output is one [64, 28] tile per core
    (3 main_logits | 3 main_probs | 22 fine).
"""

import numpy as np
import ml_dtypes

import concourse.bass as bass
import concourse.bacc as bacc
import concourse.mybir as mybir
import concourse.tile as tile
from concourse.bass_utils import run_bass_kernel_spmd

# Problem shapes (hardcoded; kernel.py must be self-contained).
B, S, H, E = 64, 512, 768, 8
HID = 256
N_CORES = 8
BPC = B // N_CORES          # batch rows per core = 8
NE = BPC * E                # entities per core = 64
TC = S // 128               # token chunks per row = 4
HC = H // 128               # h-chunks = 6
F3 = 3 * H                  # 2304
KC = F3 // 128              # feature chunks = 18
HID4 = 4 * HID              # 1024
HC2 = HID4 // 128           # 8
NOUT = 3 + 6 + 12 + 4       # 25
NCOLS = 3 + 3 + 22          # output tile columns = 28
MPAD = 32                   # mask columns: m = 4*j + seg, seg-3 lane zero
NPAI = HID4 - HID           # 768 p/a/i hidden units

F32 = mybir.dt.float32
BF16 = mybir.dt.bfloat16
DT = F32                    # sequence/mask dtype (fp32: argmax ties)

_BASS_CACHE = {}


def _build_module():
    nc = bacc.Bacc("TRN2", target_bir_lowering=False)
    seq_in = nc.dram_tensor("seq_in", [BPC, 128, TC * H], DT, kind="ExternalInput")
    mask_in = nc.dram_tensor("mask_in", [128, BPC * TC, MPAD], DT, kind="ExternalInput")
    w1m_in = nc.dram_tensor("w1m_in", [128, KC, HID], F32, kind="ExternalInput")
    w1p_in = nc.dram_tensor("w1p_in", [128, KC, NPAI], BF16, kind="ExternalInput")
    b1_in = nc.dram_tensor("b1_in", [128, HC2], F32, kind="ExternalInput")
    w2_in = nc.dram_tensor("w2_in", [128, HC2, NOUT], F32, kind="ExternalInput")
    b2_in = nc.dram_tensor("b2_in", [NE, NOUT], F32, kind="ExternalInput")
    out = nc.dram_tensor("out", [NE, NCOLS], F32, kind="ExternalOutput")

    with tile.TileContext(nc) as tc:
        _body(tc, seq_in[:], mask_in[:], w1m_in[:], w1p_in[:], b1_in[:],
              w2_in[:], b2_in[:], out[:])
    if not nc.is_finalized():
        nc.finalize()
    return nc


def _body(tc, seq_ap, mask_ap, w1m_ap, w1p_ap, b1_ap, w2_ap, b2_ap, out_ap):
    nc = tc.nc
    X = mybir.AxisListType.X
    HH = H // 2          # 384 = one fp32 psum bank
    HCH = HC // 2        # 3 h-chunks per half
    with (
        tc.tile_pool(name="singles", bufs=1) as singles,
        tc.tile_pool(name="seqrow", bufs=1) as seqrow,
        tc.tile_pool(name="epi", bufs=1) as epi,
        tc.tile_pool(name="psum1", bufs=4, space="PSUM") as psum1,
        tc.tile_pool(name="psum2", bufs=1, space="PSUM") as psum2,
        tc.tile_pool(name="psum3", bufs=1, space="PSUM") as psum3,
    ):
        # --- small resident inputs (issued first; cheap) ---------------------
        mask_sb = singles.tile([128, BPC * TC, MPAD], DT)
        nc.sync.dma_start(mask_sb, mask_ap)
        b1_sb = singles.tile([128, HC2], F32)
        nc.scalar.dma_start(b1_sb, b1_ap)
        w2_sb = singles.tile([128, HC2, NOUT], F32)
        nc.scalar.dma_start(w2_sb, w2_ap)
        b2_sb = singles.tile([NE, NOUT], F32)
        nc.scalar.dma_start(b2_sb, b2_ap)

        # X^T: [feat-in-chunk, hchunk, row, mask] -> stage-2 lhsT slices
        xt_sb = singles.tile([128, HC, BPC, MPAD], F32)
        xtb_sb = singles.tile([128, HC, BPC, MPAD], BF16)

        # --- PE warm-up: cheap dummy matmuls bridge the idle window before
        # row 0 lands so the HAM clock gate (4/8 -> 8/8) opens early.
        warm_sb = singles.tile([128, HH], F32)
        nc.vector.memset(warm_sb, 0.0)
        warm_ps = psum1.tile([MPAD, HH], F32, tag="ps")
        for _ in range(18):
            nc.tensor.matmul(warm_ps, warm_sb[:, 0:MPAD], warm_sb,
                             start=True, stop=True)
        nc.vector.tensor_copy(warm_sb[0:MPAD, 0:1], warm_ps[:, 0:1])

        # --- all input DMAs issued upfront: seq rows alternate the two HWDGE
        # rings; weights queue right behind them (pai weights last) ----------
        seq_tiles = []
        for b in range(BPC):
            seq_sb = seqrow.tile([128, TC, H], DT, name=f"seq{b}")
            eng = nc.sync if b % 2 == 0 else nc.scalar
            eng.dma_start(seq_sb, seq_ap[b].rearrange("p (t h) -> p t h", t=TC))
            seq_tiles.append(seq_sb)
        w1m_sb = singles.tile([128, KC, HID], F32)
        nc.sync.dma_start(w1m_sb, w1m_ap)
        w1p_sb = singles.tile([128, KC, NPAI], BF16)
        nc.scalar.dma_start(w1p_sb, w1p_ap)

        # --- stage 1: emb^T = mask^T @ seq (mask stationary) ----------------
        for b in range(BPC):
            seq_sb = seq_tiles[b]
            for half in range(2):
                ps = psum1.tile([MPAD, HH], F32)
                for t in range(TC):
                    nc.tensor.matmul(
                        ps,
                        mask_sb[:, b * TC + t, :],
                        seq_sb[:, t, half * HH : (half + 1) * HH],
                        start=(t == 0),
                        stop=(t == TC - 1),
                    )
                # ps[m, h'] with h' = hcw*128 + q*32 + p'; 32x32 block
                # transposes to xt[q*32+p', half*HCH+hcw, b, m]
                ps4 = ps.rearrange("m (hcw q p) -> m hcw q p", q=4, p=32)
                for q in range(4):
                    nc.vector.transpose(
                        xt_sb[q * 32 : (q + 1) * 32, half * HCH : (half + 1) * HCH, b, :],
                        ps4[:, :, q, :],
                    )
                # incremental bf16 cast for the p/a/i matmuls (ACT is idle)
                nc.scalar.copy(
                    xtb_sb[:, half * HCH : (half + 1) * HCH, b, :],
                    xt_sb[:, half * HCH : (half + 1) * HCH, b, :],
                )

        # --- stage 2: h = relu(X @ W1cat + b1), entities on partitions ------
        # lhsT slice over (b, j) for fixed (seg, hc): strides (32, 4) -> flat
        xtv = xt_sb.rearrange("p hc b (j s) -> p hc b j s", s=4)
        xtbv = xtb_sb.rearrange("p hc b (j s) -> p hc b j s", s=4)
        ps_m = psum2.tile([NE, HID], F32, name="ps_m")
        ps_pa = psum2.tile([NE, 512], F32, name="ps_pa")
        ps_pb = psum2.tile([NE, NPAI - 512], F32, name="ps_pb")

        # h-pre transposed to [hid-in-chunk, c2, entity]; relu+bias after
        htr_sb = singles.tile([128, HC2, NE], F32)

        def transpose_hpre(ps, c2_base, n_c2):
            # ps[n, u'] with u' = c2'*128 + q*32 + p' -> htr[q*32+p', c2_base+c2', n]
            for r in range(2):
                p4 = ps[r * 32 : (r + 1) * 32, :].rearrange(
                    "n (c2 q p) -> n c2 q p", q=4, p=32
                )
                for q in range(4):
                    nc.vector.transpose(
                        htr_sb[q * 32 : (q + 1) * 32, c2_base : c2_base + n_c2,
                               r * 32 : (r + 1) * 32],
                        p4[:, :, q, :],
                    )

        for c in range(KC):
            seg, hc = divmod(c, HC)
            st, sp = (c == 0), (c == KC - 1)
            nc.tensor.matmul(ps_m, xtv[:, hc, :, :, seg], w1m_sb[:, c, :],
                             start=st, stop=sp)
        transpose_hpre(ps_m, 0, 2)
        for c in range(KC):
            seg, hc = divmod(c, HC)
            st, sp = (c == 0), (c == KC - 1)
            nc.tensor.matmul(ps_pa, xtbv[:, hc, :, :, seg], w1p_sb[:, c, 0:512],
                             start=st, stop=sp)
            nc.tensor.matmul(ps_pb, xtbv[:, hc, :, :, seg], w1p_sb[:, c, 512:NPAI],
                             start=st, stop=sp)
        transpose_hpre(ps_pa, 2, 4)
        transpose_hpre(ps_pb, 6, 2)

        # relu(h + b1) per 128-hid chunk, bias per-partition, into h^T layout
        ht_sb = singles.tile([128, HC2, NE], F32)
        for c2 in range(HC2):
            nc.scalar.activation(
                ht_sb[:, c2, :],
                htr_sb[:, c2, :],
                mybir.ActivationFunctionType.Relu,
                bias=b1_sb[:, c2 : c2 + 1],
                scale=1.0,
            )

        # --- stage 3: logits = h @ W2blk + b2, entity on partitions ---------
        ps3 = psum3.tile([NE, NOUT], F32)
        for c2 in range(HC2):
            nc.tensor.matmul(
                ps3,
                ht_sb[:, c2, :],
                w2_sb[:, c2, :],
                start=(c2 == 0),
                stop=(c2 == HC2 - 1),
            )
        logits = epi.tile([NE, NOUT], F32)
        nc.vector.tensor_add(logits, ps3, b2_sb)

        # --- epilogue --------------------------------------------------------
        outsb = epi.tile([NE, NCOLS], F32)
        nc.vector.tensor_copy(outsb[:, 0:3], logits[:, 0:3])

        # softmax over the 3 main logits
        rmax = epi.tile([NE, 1], F32)
        nc.vector.reduce_max(rmax, logits[:, 0:3], axis=X)
        negmax = epi.tile([NE, 1], F32)
        nc.vector.tensor_scalar_mul(negmax, rmax, -1.0)
        exps = epi.tile([NE, 3], F32)
        nc.scalar.activation(
            exps, logits[:, 0:3], mybir.ActivationFunctionType.Exp, bias=negmax, scale=1.0
        )
        ssum = epi.tile([NE, 1], F32)
        nc.vector.reduce_sum(ssum, exps, axis=X)
        rinv = epi.tile([NE, 1], F32)
        nc.vector.reciprocal(rinv, ssum)
        nc.vector.tensor_scalar_mul(outsb[:, 3:6], exps, rinv)

        # argmax-select masks (first-max-wins, matching jnp.argmax)
        L0, L1, L2 = logits[:, 0:1], logits[:, 1:2], logits[:, 2:3]
        ge01 = epi.tile([NE, 1], F32)
        nc.vector.tensor_tensor(ge01, L0, L1, mybir.AluOpType.is_ge)
        ge02 = epi.tile([NE, 1], F32)
        nc.vector.tensor_tensor(ge02, L0, L2, mybir.AluOpType.is_ge)
        ge12 = epi.tile([NE, 1], F32)
        nc.vector.tensor_tensor(ge12, L1, L2, mybir.AluOpType.is_ge)
        is0 = epi.tile([NE, 1], F32)
        nc.vector.tensor_mul(is0, ge01, ge02)
        not01 = epi.tile([NE, 1], F32)
        nc.vector.tensor_scalar(
            not01, ge01, -1.0, 1.0, mybir.AluOpType.mult, mybir.AluOpType.add
        )
        is1 = epi.tile([NE, 1], F32)
        nc.vector.tensor_mul(is1, not01, ge12)
        is01 = epi.tile([NE, 1], F32)
        nc.vector.tensor_add(is01, is0, is1)
        is2 = epi.tile([NE, 1], F32)
        nc.vector.tensor_scalar(
            is2, is01, -1.0, 1.0, mybir.AluOpType.mult, mybir.AluOpType.add
        )

        sig = epi.tile([NE, 22], F32)
        nc.scalar.activation(sig, logits[:, 3:NOUT], mybir.ActivationFunctionType.Sigmoid)
        nc.vector.tensor_scalar_mul(outsb[:, 6:12], sig[:, 0:6], is0)
        nc.vector.tensor_scalar_mul(outsb[:, 12:24], sig[:, 6:18], is1)
        nc.vector.tensor_scalar_mul(outsb[:, 24:28], sig[:, 18:22], is2)

        nc.sync.dma_start(out_ap, outsb)


def _prepare_inputs(sequence_output, starts, ends, weights):
    """Host-side packing: masks, fused weights, per-core shards."""
    seq = np.ascontiguousarray(np.asarray(sequence_output, dtype=np.float32))
    s = np.asarray(starts).astype(np.int64)
    e = np.asarray(ends).astype(np.int64)

    t = np.arange(S)
    left_w = np.where(s > 0, 1.0 / np.maximum(s, 1), 0.0)          # [B,E]
    span_w = 1.0 / (e - s + 1)
    right_cnt = S - (e + 1)
    right_w = np.where(right_cnt > 0, 1.0 / np.maximum(right_cnt, 1), 0.0)

    lm = (t[None, None, :] < s[:, :, None]) * left_w[:, :, None]    # [B,E,S]
    sm = ((t[None, None, :] >= s[:, :, None]) & (t[None, None, :] <= e[:, :, None])) * span_w[:, :, None]
    rm = (t[None, None, :] > e[:, :, None]) * right_w[:, :, None]
    mask_bes = np.stack([lm, sm, rm], axis=1)                       # [B,3,E,S]
    # mask column order m = 4*j + seg (seg=3 lane zero-padded) so the stage-2
    # stationary slice over (b, j) flattens to a single strided free dim
    maskT4 = np.zeros((B, S, E, 4), np.float32)
    maskT4[:, :, :, :3] = mask_bes.transpose(0, 3, 2, 1)            # [B,S,E,3]
    maskT = maskT4.reshape(B, S, MPAD)

    mW1, pW1, aW1, iW1 = (np.asarray(weights[k], np.float32) for k in ("mW1", "pW1", "aW1", "iW1"))
    mb1, pb1, ab1, ib1 = (np.asarray(weights[k], np.float32) for k in ("mb1", "pb1", "ab1", "ib1"))
    mW2, pW2, aW2, iW2 = (np.asarray(weights[k], np.float32) for k in ("mW2", "pW2", "aW2", "iW2"))
    mb2, pb2, ab2, ib2 = (np.asarray(weights[k], np.float32) for k in ("mb2", "pb2", "ab2", "ib2"))

    w1m_host = np.ascontiguousarray(
        mW1.reshape(KC, 128, HID).transpose(1, 0, 2)
    )                                                               # [128, KC, 256]
    W1pai = np.concatenate([pW1, aW1, iW1], axis=1)                 # [2304, 768]
    w1p_host = np.ascontiguousarray(
        W1pai.reshape(KC, 128, NPAI).transpose(1, 0, 2).astype(ml_dtypes.bfloat16)
    )                                                               # [128, KC, 768]
    b1cat = np.concatenate([mb1, pb1, ab1, ib1]).astype(np.float32)  # [1024]
    b1_host = np.ascontiguousarray(b1cat.reshape(HC2, 128).T)        # [128, 8]

    W2blk = np.zeros((HID4, NOUT), np.float32)
    W2blk[0:256, 0:3] = mW2
    W2blk[256:512, 3:9] = pW2
    W2blk[512:768, 9:21] = aW2
    W2blk[768:1024, 21:25] = iW2
    w2_host = np.ascontiguousarray(
        W2blk.reshape(HC2, 128, NOUT).transpose(1, 0, 2)
    )                                                               # [128, 8, 25]
    b2cat = np.concatenate([mb2, pb2, ab2, ib2]).astype(np.float32)  # [25]
    b2_host = np.ascontiguousarray(np.broadcast_to(b2cat, (NE, NOUT)).copy())

    in_maps = []
    for c in range(N_CORES):
        bs = slice(c * BPC, (c + 1) * BPC)
        # [BPC, 128, TC*H]: contiguous 12KB per-partition lines
        seq_c = np.ascontiguousarray(
            seq[bs].reshape(BPC, TC, 128, H).transpose(0, 2, 1, 3).reshape(BPC, 128, TC * H)
        )
        mask_c = np.ascontiguousarray(
            maskT[bs].reshape(BPC, TC, 128, MPAD).transpose(2, 0, 1, 3)
            .reshape(128, BPC * TC, MPAD)
        )
        in_maps.append(
            {
                "seq_in": seq_c,
                "mask_in": mask_c,
                "w1m_in": w1m_host,
                "w1p_in": w1p_host,
                "b1_in": b1_host,
                "w2_in": w2_host,
                "b2_in": b2_host,
            }
        )
    return in_maps


def run(inputs, trace=False):
    """Run the kernel; returns ((main_logits, main_probs, fine), BassKernelResults)."""
    if "nc" not in _BASS_CACHE:
        _BASS_CACHE["nc"] = _build_module()
    nc = _BASS_CACHE["nc"]

    weights = {k: inputs[k] for k in inputs if k not in
               ("sequence_output", "entity_start_positions", "entity_end_positions")}
    in_maps = _prepare_inputs(
        inputs["sequence_output"],
        inputs["entity_start_positions"],
        inputs["entity_end_positions"],
        weights,
    )
    res = run_bass_kernel_spmd(nc, in_maps, core_ids=list(range(N_CORES)), trace=trace)
    allout = np.concatenate([r["out"] for r in res.results], axis=0)  # [512, 28]
    main_logits = np.ascontiguousarray(allout[:, 0:3])
    main_probs = np.ascontiguousarray(allout[:, 3:6])
    fine = np.ascontiguousarray(allout[:, 6:28])
    return (main_logits, main_probs, fine), res


def kernel(**inputs):
    outs, _ = run(inputs, trace=False)
    return outs


# revision 28
# speedup vs baseline: 1.0574x; 1.0574x over previous
"""Trainium2 Bass kernel for nn_EntityRoleClassifier (segment_reduce).

Strategy (data-parallel over batch, 8 NeuronCores):
  - Each core gets B/8 = 8 batch rows of sequence_output plus replicated MLP
    weights.
  - Per-entity left/span/right mean-pools are a dense matmul against
    host-built normalized interval masks (mask stationary, sequence moving),
    giving emb^T = mask^T @ seq in PSUM; DVE 32x32 block transposes land
    X^T (features on partitions) for the MLP.
  - The 4 head MLPs are fused: W1cat = [mW1|pW1|aW1|iW1] (2304x1024),
    W2blk = block-diagonal (1024x25).  The main head runs fp32 end-to-end
    (argmax near-ties need it); the p/a/i heads' W1 is bf16 (their sigmoids
    tolerate ~1e-4 error and it cuts weight DMA).
  - Softmax / argmax-select / sigmoid epilogue runs on-device, entity on
    partitions; output is one [64, 28] tile per core
    (3 main_logits | 3 main_probs | 22 fine).
"""

import numpy as np
import ml_dtypes

import concourse.bass as bass
import concourse.bacc as bacc
import concourse.mybir as mybir
import concourse.tile as tile
from concourse.bass_utils import run_bass_kernel_spmd

# Problem shapes (hardcoded; kernel.py must be self-contained).
B, S, H, E = 64, 512, 768, 8
HID = 256
N_CORES = 8
BPC = B // N_CORES          # batch rows per core = 8
NE = BPC * E                # entities per core = 64
TC = S // 128               # token chunks per row = 4
HC = H // 128               # h-chunks = 6
F3 = 3 * H                  # 2304
KC = F3 // 128              # feature chunks = 18
HID4 = 4 * HID              # 1024
HC2 = HID4 // 128           # 8
NOUT = 3 + 6 + 12 + 4       # 25
NCOLS = 3 + 3 + 22          # output tile columns = 28
MPAD = 32                   # mask columns: m = 4*j + seg, seg-3 lane zero
NPAI = HID4 - HID           # 768 p/a/i hidden units

F32 = mybir.dt.float32
BF16 = mybir.dt.bfloat16
DT = F32                    # sequence/mask dtype (fp32: argmax ties)

_BASS_CACHE = {}


def _build_module():
    nc = bacc.Bacc("TRN2", target_bir_lowering=False)
    seq_in = nc.dram_tensor("seq_in", [BPC, 128, TC * H], DT, kind="ExternalInput")
    mask_in = nc.dram_tensor("mask_in", [128, BPC * TC, MPAD], DT, kind="ExternalInput")
    w1m_in = nc.dram_tensor("w1m_in", [128, KC, HID], F32, kind="ExternalInput")
    w1p_in = nc.dram_tensor("w1p_in", [128, KC, NPAI], BF16, kind="ExternalInput")
    # packed smalls: [:, 0:8] b1 (hid-chunk layout), [:, 8:208] W2blk,
    # [0:64, 208:233] b2 broadcast
    packed_in = nc.dram_tensor("packed_in", [128, HC2 + HC2 * NOUT + NOUT], F32,
                               kind="ExternalInput")
    out = nc.dram_tensor("out", [NE, NCOLS], F32, kind="ExternalOutput")

    with tile.TileContext(nc) as tc:
        _body(tc, seq_in[:], mask_in[:], w1m_in[:], w1p_in[:], packed_in[:], out[:])
    if not nc.is_finalized():
        nc.finalize()
    return nc


def _body(tc, seq_ap, mask_ap, w1m_ap, w1p_ap, packed_ap, out_ap):
    nc = tc.nc
    X = mybir.AxisListType.X
    HH = H // 2          # 384 = one fp32 psum bank
    HCH = HC // 2        # 3 h-chunks per half
    with (
        tc.tile_pool(name="singles", bufs=1) as singles,
        tc.tile_pool(name="seqrow", bufs=1) as seqrow,
        tc.tile_pool(name="epi", bufs=1) as epi,
        tc.tile_pool(name="psum1", bufs=4, space="PSUM") as psum1,
        tc.tile_pool(name="psum2", bufs=1, space="PSUM") as psum2,
        tc.tile_pool(name="psum3", bufs=1, space="PSUM") as psum3,
    ):
        # --- all input DMAs, in lane-friendly order: 8 HWDGE semaphore lanes
        # round-robin in emission order, so DMAs 9..11 reuse lanes whose first
        # user finished long before (mask / row 0 / row 1).  Rows 0-3 are
        # single DMAs (prompt availability), rows 4-7 paired; smalls packed.
        mask_sb = singles.tile([128, BPC * TC, MPAD], DT)
        nc.sync.dma_start(mask_sb, mask_ap)                       # lane 0, SP
        seq_tiles = [None] * BPC
        for b in range(4):                                        # lanes 1-4
            t_ = seqrow.tile([128, TC, H], DT, name=f"seq{b}")
            eng = nc.sync if b % 2 == 0 else nc.scalar
            eng.dma_start(t_, seq_ap[b].rearrange("p (t h) -> p t h", t=TC))
            seq_tiles[b] = t_
        pair45 = seqrow.tile([128, 2, TC, H], DT, name="seq45")   # lane 5, SP
        nc.sync.dma_start(pair45, seq_ap[4:6].rearrange("b p (t h) -> p b t h", t=TC))
        pair67 = seqrow.tile([128, 2, TC, H], DT, name="seq67")   # lane 6, ACT
        nc.scalar.dma_start(pair67, seq_ap[6:8].rearrange("b p (t h) -> p b t h", t=TC))
        for k in range(2):
            seq_tiles[4 + k] = pair45[:, k]
            seq_tiles[6 + k] = pair67[:, k]
        w1m_sb = singles.tile([128, KC, HID], F32)
        nc.sync.dma_start(w1m_sb, w1m_ap)                         # lane 7, SP
        w1p_sb = singles.tile([128, KC, NPAI], BF16)
        nc.scalar.dma_start(w1p_sb, w1p_ap)                       # lane 0 reuse
        packed_sb = singles.tile([128, HC2 + HC2 * NOUT + NOUT], F32)
        nc.scalar.dma_start(packed_sb, packed_ap)                 # lane 1 reuse
        b1_sb = packed_sb[:, 0:HC2]
        w2_sb = packed_sb[:, HC2 : HC2 + HC2 * NOUT].rearrange(
            "p (c n) -> p c n", n=NOUT
        )
        b2_sb = packed_sb[0:NE, HC2 + HC2 * NOUT :]

        # X^T: [feat-in-chunk, hchunk, row, mask] -> stage-2 lhsT slices
        xt_sb = singles.tile([128, HC, BPC, MPAD], F32)
        xtb_sb = singles.tile([128, HC, BPC, MPAD], BF16)

        # --- PE warm-up bridging until row 0 lands (HAM 4/8 -> 8/8), plus
        # ACT-table preloads so Exp/Sigmoid tables aren't loaded mid-epilogue
        warm_sb = singles.tile([128, HH], F32)
        nc.vector.memset(warm_sb, 0.0)
        warm_ps = psum1.tile([MPAD, HH], F32, tag="ps")
        for _ in range(8):
            nc.tensor.matmul(warm_ps, warm_sb[:, 0:MPAD], warm_sb,
                             start=True, stop=True)
        nc.vector.tensor_copy(warm_sb[0:MPAD, 0:1], warm_ps[:, 0:1])
        scrap_sb = singles.tile([1, 2], F32)
        nc.scalar.activation(scrap_sb[:, 0:1], warm_sb[0:1, 0:1],
                             mybir.ActivationFunctionType.Exp)
        nc.scalar.activation(scrap_sb[:, 1:2], warm_sb[0:1, 0:1],
                             mybir.ActivationFunctionType.Sigmoid)

        # --- stage 1: emb^T = mask^T @ seq (mask stationary) ----------------
        for b in range(BPC):
            seq_sb = seq_tiles[b]
            for half in range(2):
                ps = psum1.tile([MPAD, HH], F32)
                for t in range(TC):
                    nc.tensor.matmul(
                        ps,
                        mask_sb[:, b * TC + t, :],
                        seq_sb[:, t, half * HH : (half + 1) * HH],
                        start=(t == 0),
                        stop=(t == TC - 1),
                    )
                # ps[m, h'] with h' = hcw*128 + q*32 + p'; 32x32 block
                # transposes to xt[q*32+p', half*HCH+hcw, b, m]
                ps4 = ps.rearrange("m (hcw q p) -> m hcw q p", q=4, p=32)
                for q in range(4):
                    nc.vector.transpose(
                        xt_sb[q * 32 : (q + 1) * 32, half * HCH : (half + 1) * HCH, b, :],
                        ps4[:, :, q, :],
                    )
                # incremental bf16 cast for the p/a/i matmuls (ACT is idle)
                nc.scalar.copy(
                    xtb_sb[:, half * HCH : (half + 1) * HCH, b, :],
                    xt_sb[:, half * HCH : (half + 1) * HCH, b, :],
                )

        # --- stage 2: h = relu(X @ W1cat + b1), entities on partitions ------
        # lhsT slice over (b, j) for fixed (seg, hc): strides (32, 4) -> flat
        xtv = xt_sb.rearrange("p hc b (j s) -> p hc b j s", s=4)
        xtbv = xtb_sb.rearrange("p hc b (j s) -> p hc b j s", s=4)
        ps_m = psum2.tile([NE, HID], F32, name="ps_m")
        ps_pa = psum2.tile([NE, 512], F32, name="ps_pa")
        ps_pb = psum2.tile([NE, NPAI - 512], F32, name="ps_pb")

        # h-pre transposed to [hid-in-chunk, c2, entity]; relu+bias after
        htr_sb = singles.tile([128, HC2, NE], F32)

        def transpose_hpre(ps, c2_base, n_c2):
            # ps[n, u'] with u' = c2'*128 + q*32 + p' -> htr[q*32+p', c2_base+c2', n]
            for r in range(2):
                p4 = ps[r * 32 : (r + 1) * 32, :].rearrange(
                    "n (c2 q p) -> n c2 q p", q=4, p=32
                )
                for q in range(4):
                    nc.vector.transpose(
                        htr_sb[q * 32 : (q + 1) * 32, c2_base : c2_base + n_c2,
                               r * 32 : (r + 1) * 32],
                        p4[:, :, q, :],
                    )

        for c in range(KC):
            seg, hc = divmod(c, HC)
            st, sp = (c == 0), (c == KC - 1)
            nc.tensor.matmul(ps_m, xtv[:, hc, :, :, seg], w1m_sb[:, c, :],
                             start=st, stop=sp)
        transpose_hpre(ps_m, 0, 2)
        for c in range(KC):
            seg, hc = divmod(c, HC)
            st, sp = (c == 0), (c == KC - 1)
            nc.tensor.matmul(ps_pa, xtbv[:, hc, :, :, seg], w1p_sb[:, c, 0:512],
                             start=st, stop=sp)
            nc.tensor.matmul(ps_pb, xtbv[:, hc, :, :, seg], w1p_sb[:, c, 512:NPAI],
                             start=st, stop=sp)
        transpose_hpre(ps_pa, 2, 4)
        transpose_hpre(ps_pb, 6, 2)

        # relu(h + b1) per 128-hid chunk, bias per-partition, into h^T layout
        ht_sb = singles.tile([128, HC2, NE], F32)
        for c2 in range(HC2):
            nc.scalar.activation(
                ht_sb[:, c2, :],
                htr_sb[:, c2, :],
                mybir.ActivationFunctionType.Relu,
                bias=b1_sb[:, c2 : c2 + 1],
                scale=1.0,
            )

        # --- stage 3: logits = h @ W2blk + b2, entity on partitions ---------
        ps3 = psum3.tile([NE, NOUT], F32)
        for c2 in range(HC2):
            nc.tensor.matmul(
                ps3,
                ht_sb[:, c2, :],
                w2_sb[:, c2, :],
                start=(c2 == 0),
                stop=(c2 == HC2 - 1),
            )
        logits = epi.tile([NE, NOUT], F32)
        nc.vector.tensor_add(logits, ps3, b2_sb)

        # --- epilogue --------------------------------------------------------
        outsb = epi.tile([NE, NCOLS], F32)
        nc.vector.tensor_copy(outsb[:, 0:3], logits[:, 0:3])

        # softmax over the 3 main logits
        rmax = epi.tile([NE, 1], F32)
        nc.vector.reduce_max(rmax, logits[:, 0:3], axis=X)
        negmax = epi.tile([NE, 1], F32)
        nc.vector.tensor_scalar_mul(negmax, rmax, -1.0)
        exps = epi.tile([NE, 3], F32)
        nc.scalar.activation(
            exps, logits[:, 0:3], mybir.ActivationFunctionType.Exp, bias=negmax, scale=1.0
        )
        ssum = epi.tile([NE, 1], F32)
        nc.vector.reduce_sum(ssum, exps, axis=X)
        rinv = epi.tile([NE, 1], F32)
        nc.vector.reciprocal(rinv, ssum)
        nc.vector.tensor_scalar_mul(outsb[:, 3:6], exps, rinv)

        # argmax-select masks (first-max-wins, matching jnp.argmax)
        L0, L1, L2 = logits[:, 0:1], logits[:, 1:2], logits[:, 2:3]
        ge01 = epi.tile([NE, 1], F32)
        nc.vector.tensor_tensor(ge01, L0, L1, mybir.AluOpType.is_ge)
        ge02 = epi.tile([NE, 1], F32)
        nc.vector.tensor_tensor(ge02, L0, L2, mybir.AluOpType.is_ge)
        ge12 = epi.tile([NE, 1], F32)
        nc.vector.tensor_tensor(ge12, L1, L2, mybir.AluOpType.is_ge)
        is0 = epi.tile([NE, 1], F32)
        nc.vector.tensor_mul(is0, ge01, ge02)
        not01 = epi.tile([NE, 1], F32)
        nc.vector.tensor_scalar(
            not01, ge01, -1.0, 1.0, mybir.AluOpType.mult, mybir.AluOpType.add
        )
        is1 = epi.tile([NE, 1], F32)
        nc.vector.tensor_mul(is1, not01, ge12)
        is01 = epi.tile([NE, 1], F32)
        nc.vector.tensor_add(is01, is0, is1)
        is2 = epi.tile([NE, 1], F32)
        nc.vector.tensor_scalar(
            is2, is01, -1.0, 1.0, mybir.AluOpType.mult, mybir.AluOpType.add
        )

        sig = epi.tile([NE, 22], F32)
        nc.scalar.activation(sig, logits[:, 3:NOUT], mybir.ActivationFunctionType.Sigmoid)
        nc.vector.tensor_scalar_mul(outsb[:, 6:12], sig[:, 0:6], is0)
        nc.vector.tensor_scalar_mul(outsb[:, 12:24], sig[:, 6:18], is1)
        nc.vector.tensor_scalar_mul(outsb[:, 24:28], sig[:, 18:22], is2)

        nc.sync.dma_start(out_ap, outsb)


def _prepare_inputs(sequence_output, starts, ends, weights):
    """Host-side packing: masks, fused weights, per-core shards."""
    seq = np.ascontiguousarray(np.asarray(sequence_output, dtype=np.float32))
    s = np.asarray(starts).astype(np.int64)
    e = np.asarray(ends).astype(np.int64)

    t = np.arange(S)
    left_w = np.where(s > 0, 1.0 / np.maximum(s, 1), 0.0)          # [B,E]
    span_w = 1.0 / (e - s + 1)
    right_cnt = S - (e + 1)
    right_w = np.where(right_cnt > 0, 1.0 / np.maximum(right_cnt, 1), 0.0)

    lm = (t[None, None, :] < s[:, :, None]) * left_w[:, :, None]    # [B,E,S]
    sm = ((t[None, None, :] >= s[:, :, None]) & (t[None, None, :] <= e[:, :, None])) * span_w[:, :, None]
    rm = (t[None, None, :] > e[:, :, None]) * right_w[:, :, None]
    mask_bes = np.stack([lm, sm, rm], axis=1)                       # [B,3,E,S]
    # mask column order m = 4*j + seg (seg=3 lane zero-padded) so the stage-2
    # stationary slice over (b, j) flattens to a single strided free dim
    maskT4 = np.zeros((B, S, E, 4), np.float32)
    maskT4[:, :, :, :3] = mask_bes.transpose(0, 3, 2, 1)            # [B,S,E,3]
    maskT = maskT4.reshape(B, S, MPAD)

    mW1, pW1, aW1, iW1 = (np.asarray(weights[k], np.float32) for k in ("mW1", "pW1", "aW1", "iW1"))
    mb1, pb1, ab1, ib1 = (np.asarray(weights[k], np.float32) for k in ("mb1", "pb1", "ab1", "ib1"))
    mW2, pW2, aW2, iW2 = (np.asarray(weights[k], np.float32) for k in ("mW2", "pW2", "aW2", "iW2"))
    mb2, pb2, ab2, ib2 = (np.asarray(weights[k], np.float32) for k in ("mb2", "pb2", "ab2", "ib2"))

    w1m_host = np.ascontiguousarray(
        mW1.reshape(KC, 128, HID).transpose(1, 0, 2)
    )                                                               # [128, KC, 256]
    W1pai = np.concatenate([pW1, aW1, iW1], axis=1)                 # [2304, 768]
    w1p_host = np.ascontiguousarray(
        W1pai.reshape(KC, 128, NPAI).transpose(1, 0, 2).astype(ml_dtypes.bfloat16)
    )                                                               # [128, KC, 768]
    b1cat = np.concatenate([mb1, pb1, ab1, ib1]).astype(np.float32)  # [1024]
    b1_host = b1cat.reshape(HC2, 128).T                              # [128, 8]

    W2blk = np.zeros((HID4, NOUT), np.float32)
    W2blk[0:256, 0:3] = mW2
    W2blk[256:512, 3:9] = pW2
    W2blk[512:768, 9:21] = aW2
    W2blk[768:1024, 21:25] = iW2
    w2_host = W2blk.reshape(HC2, 128, NOUT).transpose(1, 0, 2)       # [128, 8, 25]
    b2cat = np.concatenate([mb2, pb2, ab2, ib2]).astype(np.float32)  # [25]

    packed_host = np.zeros((128, HC2 + HC2 * NOUT + NOUT), np.float32)
    packed_host[:, 0:HC2] = b1_host
    packed_host[:, HC2 : HC2 + HC2 * NOUT] = w2_host.reshape(128, HC2 * NOUT)
    packed_host[0:NE, HC2 + HC2 * NOUT :] = np.broadcast_to(b2cat, (NE, NOUT))

    in_maps = []
    for c in range(N_CORES):
        bs = slice(c * BPC, (c + 1) * BPC)
        # [BPC, 128, TC*H]: contiguous 12KB per-partition lines
        seq_c = np.ascontiguousarray(
            seq[bs].reshape(BPC, TC, 128, H).transpose(0, 2, 1, 3).reshape(BPC, 128, TC * H)
        )
        mask_c = np.ascontiguousarray(
            maskT[bs].reshape(BPC, TC, 128, MPAD).transpose(2, 0, 1, 3)
            .reshape(128, BPC * TC, MPAD)
        )
        in_maps.append(
            {
                "seq_in": seq_c,
                "mask_in": mask_c,
                "w1m_in": w1m_host,
                "w1p_in": w1p_host,
                "packed_in": packed_host,
            }
        )
    return in_maps


def run(inputs, trace=False):
    """Run the kernel; returns ((main_logits, main_probs, fine), BassKernelResults)."""
    if "nc" not in _BASS_CACHE:
        _BASS_CACHE["nc"] = _build_module()
    nc = _BASS_CACHE["nc"]

    weights = {k: inputs[k] for k in inputs if k not in
               ("sequence_output", "entity_start_positions", "entity_end_positions")}
    in_maps = _prepare_inputs(
        inputs["sequence_output"],
        inputs["entity_start_positions"],
        inputs["entity_end_positions"],
        weights,
    )
    res = run_bass_kernel_spmd(nc, in_maps, core_ids=list(range(N_CORES)), trace=trace)
    allout = np.concatenate([r["out"] for r in res.results], axis=0)  # [512, 28]
    main_logits = np.ascontiguousarray(allout[:, 0:3])
    main_probs = np.ascontiguousarray(allout[:, 3:6])
    fine = np.ascontiguousarray(allout[:, 6:28])
    return (main_logits, main_probs, fine), res


def kernel(**inputs):
    outs, _ = run(inputs, trace=False)
    return outs


# revision 31
# speedup vs baseline: 1.0888x; 1.0297x over previous
"""Trainium2 Bass kernel for nn_EntityRoleClassifier (segment_reduce).

Strategy (data-parallel over batch, 8 NeuronCores):
  - Each core gets B/8 = 8 batch rows of sequence_output plus replicated MLP
    weights.
  - Per-entity left/span/right mean-pools are a dense matmul against
    host-built normalized interval masks (mask stationary, sequence moving),
    giving emb^T = mask^T @ seq in PSUM; DVE 32x32 block transposes land
    X^T (features on partitions) for the MLP.
  - The 4 head MLPs are fused: W1cat = [mW1|pW1|aW1|iW1] (2304x1024),
    W2blk = block-diagonal (1024x25).  The main head runs fp32 end-to-end
    (argmax near-ties need it); the p/a/i heads' W1 is bf16 (their sigmoids
    tolerate ~1e-4 error and it cuts weight DMA).
  - Softmax / argmax-select / sigmoid epilogue runs on-device, entity on
    partitions; output is one [64, 28] tile per core
    (3 main_logits | 3 main_probs | 22 fine).
"""

import numpy as np
import ml_dtypes

import concourse.bass as bass
import concourse.bacc as bacc
import concourse.mybir as mybir
import concourse.tile as tile
from concourse.bass_utils import run_bass_kernel_spmd

# Problem shapes (hardcoded; kernel.py must be self-contained).
B, S, H, E = 64, 512, 768, 8
HID = 256
N_CORES = 8
BPC = B // N_CORES          # batch rows per core = 8
NE = BPC * E                # entities per core = 64
TC = S // 128               # token chunks per row = 4
HC = H // 128               # h-chunks = 6
F3 = 3 * H                  # 2304
KC = F3 // 128              # feature chunks = 18
HID4 = 4 * HID              # 1024
HC2 = HID4 // 128           # 8
NOUT = 3 + 6 + 12 + 4       # 25
NCOLS = 3 + 3 + 22          # output tile columns = 28
MPAD = 32                   # mask columns: m = 4*j + seg, seg-3 lane zero
NPAI = HID4 - HID           # 768 p/a/i hidden units

F32 = mybir.dt.float32
BF16 = mybir.dt.bfloat16
DT = F32                    # sequence/mask dtype (fp32: argmax ties)

_BASS_CACHE = {}


def _build_module():
    nc = bacc.Bacc("TRN2", target_bir_lowering=False)
    seq_in = nc.dram_tensor("seq_in", [BPC, 128, TC * H], DT, kind="ExternalInput")
    mask_in = nc.dram_tensor("mask_in", [128, BPC * TC, MPAD], DT, kind="ExternalInput")
    w1m_in = nc.dram_tensor("w1m_in", [128, KC, HID], F32, kind="ExternalInput")
    w1p_in = nc.dram_tensor("w1p_in", [128, KC, NPAI], BF16, kind="ExternalInput")
    # packed smalls: [:, 0:8] b1 (hid-chunk layout), [:, 8:208] W2blk,
    # [0:64, 208:233] b2 broadcast
    packed_in = nc.dram_tensor("packed_in", [128, HC2 + HC2 * NOUT + NOUT], F32,
                               kind="ExternalInput")
    out = nc.dram_tensor("out", [NE, NCOLS], F32, kind="ExternalOutput")

    with tile.TileContext(nc) as tc:
        _body(tc, seq_in[:], mask_in[:], w1m_in[:], w1p_in[:], packed_in[:], out[:])
    if not nc.is_finalized():
        nc.finalize()
    return nc


def _body(tc, seq_ap, mask_ap, w1m_ap, w1p_ap, packed_ap, out_ap):
    nc = tc.nc
    X = mybir.AxisListType.X
    HH = H // 2          # 384 = one fp32 psum bank
    HCH = HC // 2        # 3 h-chunks per half
    with (
        tc.tile_pool(name="singles", bufs=1) as singles,
        tc.tile_pool(name="seqrow", bufs=1) as seqrow,
        tc.tile_pool(name="epi", bufs=1) as epi,
        tc.tile_pool(name="psum1", bufs=4, space="PSUM") as psum1,
        tc.tile_pool(name="psum2", bufs=1, space="PSUM") as psum2,
        tc.tile_pool(name="psum3", bufs=1, space="PSUM") as psum3,
    ):
        # --- all input DMAs, in lane-friendly order: 8 HWDGE semaphore lanes
        # round-robin in emission order, so DMAs 9..11 reuse lanes whose first
        # user finished long before (mask / row 0 / row 1).  Rows 0-3 are
        # single DMAs (prompt availability), rows 4-7 paired; smalls packed.
        seq_tiles = [None] * BPC
        seq_tiles[0] = seqrow.tile([128, TC, H], DT, name="seq0")  # lane 0, SP
        nc.sync.dma_start(seq_tiles[0], seq_ap[0].rearrange("p (t h) -> p t h", t=TC))
        mask_sb = singles.tile([128, BPC * TC, MPAD], DT)
        nc.scalar.dma_start(mask_sb, mask_ap)                     # lane 1, ACT
        for b in (1, 2, 3):                                       # lanes 2-4
            t_ = seqrow.tile([128, TC, H], DT, name=f"seq{b}")
            eng = nc.scalar if b % 2 == 1 else nc.sync
            eng.dma_start(t_, seq_ap[b].rearrange("p (t h) -> p t h", t=TC))
            seq_tiles[b] = t_
        pair45 = seqrow.tile([128, 2, TC, H], DT, name="seq45")   # lane 5, SP
        nc.sync.dma_start(pair45, seq_ap[4:6].rearrange("b p (t h) -> p b t h", t=TC))
        pair67 = seqrow.tile([128, 2, TC, H], DT, name="seq67")   # lane 6, ACT
        nc.scalar.dma_start(pair67, seq_ap[6:8].rearrange("b p (t h) -> p b t h", t=TC))
        for k in range(2):
            seq_tiles[4 + k] = pair45[:, k]
            seq_tiles[6 + k] = pair67[:, k]
        w1m_sb = singles.tile([128, KC, HID], F32)
        nc.sync.dma_start(w1m_sb, w1m_ap)                         # lane 7, SP
        w1p_sb = singles.tile([128, KC, NPAI], BF16)
        nc.scalar.dma_start(w1p_sb, w1p_ap)                       # lane 0 reuse
        packed_sb = singles.tile([128, HC2 + HC2 * NOUT + NOUT], F32)
        nc.scalar.dma_start(packed_sb, packed_ap)                 # lane 1 reuse
        b1_sb = packed_sb[:, 0:HC2]
        w2_sb = packed_sb[:, HC2 : HC2 + HC2 * NOUT].rearrange(
            "p (c n) -> p c n", n=NOUT
        )
        b2_sb = packed_sb[0:NE, HC2 + HC2 * NOUT :]

        # X^T: [feat-in-chunk, hchunk, row, mask] -> stage-2 lhsT slices
        xt_sb = singles.tile([128, HC, BPC, MPAD], F32)
        xtb_sb = singles.tile([128, HC, BPC, MPAD], BF16)

        # --- PE warm-up bridging until row 0 lands (HAM 4/8 -> 8/8), plus
        # ACT-table preloads so Exp/Sigmoid tables aren't loaded mid-epilogue
        warm_sb = singles.tile([128, HH], F32)
        nc.vector.memset(warm_sb, 0.0)
        warm_ps = psum1.tile([MPAD, HH], F32, tag="ps")
        for _ in range(8):
            nc.tensor.matmul(warm_ps, warm_sb[:, 0:MPAD], warm_sb,
                             start=True, stop=True)
        nc.vector.tensor_copy(warm_sb[0:MPAD, 0:1], warm_ps[:, 0:1])
        scrap_sb = singles.tile([1, 2], F32)
        nc.scalar.activation(scrap_sb[:, 0:1], warm_sb[0:1, 0:1],
                             mybir.ActivationFunctionType.Exp)
        nc.scalar.activation(scrap_sb[:, 1:2], warm_sb[0:1, 0:1],
                             mybir.ActivationFunctionType.Sigmoid)

        # --- stage 1: emb^T = mask^T @ seq (mask stationary) ----------------
        for b in range(BPC):
            seq_sb = seq_tiles[b]
            for half in range(2):
                ps = psum1.tile([MPAD, HH], F32)
                for t in range(TC):
                    nc.tensor.matmul(
                        ps,
                        mask_sb[:, b * TC + t, :],
                        seq_sb[:, t, half * HH : (half + 1) * HH],
                        start=(t == 0),
                        stop=(t == TC - 1),
                    )
                # ps[m, h'] with h' = hcw*128 + q*32 + p'; 32x32 block
                # transposes to xt[q*32+p', half*HCH+hcw, b, m]
                ps4 = ps.rearrange("m (hcw q p) -> m hcw q p", q=4, p=32)
                for q in range(4):
                    nc.vector.transpose(
                        xt_sb[q * 32 : (q + 1) * 32, half * HCH : (half + 1) * HCH, b, :],
                        ps4[:, :, q, :],
                    )
                # incremental bf16 cast for the p/a/i matmuls (ACT is idle)
                nc.scalar.copy(
                    xtb_sb[:, half * HCH : (half + 1) * HCH, b, :],
                    xt_sb[:, half * HCH : (half + 1) * HCH, b, :],
                )

        # --- stage 2: h = relu(X @ W1cat + b1), entities on partitions ------
        # lhsT slice over (b, j) for fixed (seg, hc): strides (32, 4) -> flat
        xtv = xt_sb.rearrange("p hc b (j s) -> p hc b j s", s=4)
        xtbv = xtb_sb.rearrange("p hc b (j s) -> p hc b j s", s=4)
        ps_m = psum2.tile([NE, HID], F32, name="ps_m")
        ps_pa = psum2.tile([NE, 512], F32, name="ps_pa")
        ps_pb = psum2.tile([NE, NPAI - 512], F32, name="ps_pb")

        # h-pre transposed to [hid-in-chunk, c2, entity]; relu+bias after
        htr_sb = singles.tile([128, HC2, NE], F32)

        def transpose_hpre(ps, c2_base, n_c2):
            # ps[n, u'] with u' = c2'*128 + q*32 + p' -> htr[q*32+p', c2_base+c2', n]
            for r in range(2):
                p4 = ps[r * 32 : (r + 1) * 32, :].rearrange(
                    "n (c2 q p) -> n c2 q p", q=4, p=32
                )
                for q in range(4):
                    nc.vector.transpose(
                        htr_sb[q * 32 : (q + 1) * 32, c2_base : c2_base + n_c2,
                               r * 32 : (r + 1) * 32],
                        p4[:, :, q, :],
                    )

        for c in range(KC):
            seg, hc = divmod(c, HC)
            st, sp = (c == 0), (c == KC - 1)
            nc.tensor.matmul(ps_m, xtv[:, hc, :, :, seg], w1m_sb[:, c, :],
                             start=st, stop=sp)
        transpose_hpre(ps_m, 0, 2)
        for c in range(KC):
            seg, hc = divmod(c, HC)
            st, sp = (c == 0), (c == KC - 1)
            nc.tensor.matmul(ps_pa, xtbv[:, hc, :, :, seg], w1p_sb[:, c, 0:512],
                             start=st, stop=sp)
            nc.tensor.matmul(ps_pb, xtbv[:, hc, :, :, seg], w1p_sb[:, c, 512:NPAI],
                             start=st, stop=sp)
        transpose_hpre(ps_pa, 2, 4)
        transpose_hpre(ps_pb, 6, 2)

        # relu(h + b1) per 128-hid chunk, bias per-partition, into h^T layout
        ht_sb = singles.tile([128, HC2, NE], F32)
        for c2 in range(HC2):
            nc.scalar.activation(
                ht_sb[:, c2, :],
                htr_sb[:, c2, :],
                mybir.ActivationFunctionType.Relu,
                bias=b1_sb[:, c2 : c2 + 1],
                scale=1.0,
            )

        # --- stage 3: logits = h @ W2blk + b2, entity on partitions ---------
        # (c2 6,7 last: their h^T chunks come off the final pai psum, so the
        # earlier chunks' matmuls overlap those transposes)
        ps3 = psum3.tile([NE, NOUT], F32)
        for i, c2 in enumerate([0, 1, 2, 3, 4, 5, 6, 7]):
            nc.tensor.matmul(
                ps3,
                ht_sb[:, c2, :],
                w2_sb[:, c2, :],
                start=(i == 0),
                stop=(i == HC2 - 1),
            )
        logits = epi.tile([NE, NOUT], F32)
        nc.vector.tensor_add(logits, ps3, b2_sb)

        # --- epilogue --------------------------------------------------------
        outsb = epi.tile([NE, NCOLS], F32)
        nc.vector.tensor_copy(outsb[:, 0:3], logits[:, 0:3])

        # softmax over the 3 main logits
        rmax = epi.tile([NE, 1], F32)
        nc.vector.reduce_max(rmax, logits[:, 0:3], axis=X)
        negmax = epi.tile([NE, 1], F32)
        nc.vector.tensor_scalar_mul(negmax, rmax, -1.0)
        exps = epi.tile([NE, 3], F32)
        ssum = epi.tile([NE, 1], F32)
        nc.scalar.activation(
            exps, logits[:, 0:3], mybir.ActivationFunctionType.Exp,
            bias=negmax, scale=1.0, accum_out=ssum,
        )
        rinv = epi.tile([NE, 1], F32)
        nc.vector.reciprocal(rinv, ssum)
        nc.vector.tensor_scalar_mul(outsb[:, 3:6], exps, rinv)

        # argmax-select masks (first-max-wins, matching jnp.argmax)
        L0, L1, L2 = logits[:, 0:1], logits[:, 1:2], logits[:, 2:3]
        ge01 = epi.tile([NE, 1], F32)
        nc.vector.tensor_tensor(ge01, L0, L1, mybir.AluOpType.is_ge)
        ge02 = epi.tile([NE, 1], F32)
        nc.vector.tensor_tensor(ge02, L0, L2, mybir.AluOpType.is_ge)
        ge12 = epi.tile([NE, 1], F32)
        nc.vector.tensor_tensor(ge12, L1, L2, mybir.AluOpType.is_ge)
        is0 = epi.tile([NE, 1], F32)
        nc.vector.tensor_mul(is0, ge01, ge02)
        not01 = epi.tile([NE, 1], F32)
        nc.vector.tensor_scalar(
            not01, ge01, -1.0, 1.0, mybir.AluOpType.mult, mybir.AluOpType.add
        )
        is1 = epi.tile([NE, 1], F32)
        nc.vector.tensor_mul(is1, not01, ge12)
        is01 = epi.tile([NE, 1], F32)
        nc.vector.tensor_add(is01, is0, is1)
        is2 = epi.tile([NE, 1], F32)
        nc.vector.tensor_scalar(
            is2, is01, -1.0, 1.0, mybir.AluOpType.mult, mybir.AluOpType.add
        )

        sig = epi.tile([NE, 22], F32)
        nc.scalar.activation(sig, logits[:, 3:NOUT], mybir.ActivationFunctionType.Sigmoid)
        nc.vector.tensor_scalar_mul(outsb[:, 6:12], sig[:, 0:6], is0)
        nc.vector.tensor_scalar_mul(outsb[:, 12:24], sig[:, 6:18], is1)
        nc.vector.tensor_scalar_mul(outsb[:, 24:28], sig[:, 18:22], is2)

        nc.sync.dma_start(out_ap, outsb)


def _prepare_inputs(sequence_output, starts, ends, weights):
    """Host-side packing: masks, fused weights, per-core shards."""
    seq = np.ascontiguousarray(np.asarray(sequence_output, dtype=np.float32))
    s = np.asarray(starts).astype(np.int64)
    e = np.asarray(ends).astype(np.int64)

    t = np.arange(S)
    left_w = np.where(s > 0, 1.0 / np.maximum(s, 1), 0.0)          # [B,E]
    span_w = 1.0 / (e - s + 1)
    right_cnt = S - (e + 1)
    right_w = np.where(right_cnt > 0, 1.0 / np.maximum(right_cnt, 1), 0.0)

    lm = (t[None, None, :] < s[:, :, None]) * left_w[:, :, None]    # [B,E,S]
    sm = ((t[None, None, :] >= s[:, :, None]) & (t[None, None, :] <= e[:, :, None])) * span_w[:, :, None]
    rm = (t[None, None, :] > e[:, :, None]) * right_w[:, :, None]
    mask_bes = np.stack([lm, sm, rm], axis=1)                       # [B,3,E,S]
    # mask column order m = 4*j + seg (seg=3 lane zero-padded) so the stage-2
    # stationary slice over (b, j) flattens to a single strided free dim
    maskT4 = np.zeros((B, S, E, 4), np.float32)
    maskT4[:, :, :, :3] = mask_bes.transpose(0, 3, 2, 1)            # [B,S,E,3]
    maskT = maskT4.reshape(B, S, MPAD)

    mW1, pW1, aW1, iW1 = (np.asarray(weights[k], np.float32) for k in ("mW1", "pW1", "aW1", "iW1"))
    mb1, pb1, ab1, ib1 = (np.asarray(weights[k], np.float32) for k in ("mb1", "pb1", "ab1", "ib1"))
    mW2, pW2, aW2, iW2 = (np.asarray(weights[k], np.float32) for k in ("mW2", "pW2", "aW2", "iW2"))
    mb2, pb2, ab2, ib2 = (np.asarray(weights[k], np.float32) for k in ("mb2", "pb2", "ab2", "ib2"))

    w1m_host = np.ascontiguousarray(
        mW1.reshape(KC, 128, HID).transpose(1, 0, 2)
    )                                                               # [128, KC, 256]
    W1pai = np.concatenate([pW1, aW1, iW1], axis=1)                 # [2304, 768]
    w1p_host = np.ascontiguousarray(
        W1pai.reshape(KC, 128, NPAI).transpose(1, 0, 2).astype(ml_dtypes.bfloat16)
    )                                                               # [128, KC, 768]
    b1cat = np.concatenate([mb1, pb1, ab1, ib1]).astype(np.float32)  # [1024]
    b1_host = b1cat.reshape(HC2, 128).T                              # [128, 8]

    W2blk = np.zeros((HID4, NOUT), np.float32)
    W2blk[0:256, 0:3] = mW2
    W2blk[256:512, 3:9] = pW2
    W2blk[512:768, 9:21] = aW2
    W2blk[768:1024, 21:25] = iW2
    w2_host = W2blk.reshape(HC2, 128, NOUT).transpose(1, 0, 2)       # [128, 8, 25]
    b2cat = np.concatenate([mb2, pb2, ab2, ib2]).astype(np.float32)  # [25]

    packed_host = np.zeros((128, HC2 + HC2 * NOUT + NOUT), np.float32)
    packed_host[:, 0:HC2] = b1_host
    packed_host[:, HC2 : HC2 + HC2 * NOUT] = w2_host.reshape(128, HC2 * NOUT)
    packed_host[0:NE, HC2 + HC2 * NOUT :] = np.broadcast_to(b2cat, (NE, NOUT))

    in_maps = []
    for c in range(N_CORES):
        bs = slice(c * BPC, (c + 1) * BPC)
        # [BPC, 128, TC*H]: contiguous 12KB per-partition lines
        seq_c = np.ascontiguousarray(
            seq[bs].reshape(BPC, TC, 128, H).transpose(0, 2, 1, 3).reshape(BPC, 128, TC * H)
        )
        mask_c = np.ascontiguousarray(
            maskT[bs].reshape(BPC, TC, 128, MPAD).transpose(2, 0, 1, 3)
            .reshape(128, BPC * TC, MPAD)
        )
        in_maps.append(
            {
                "seq_in": seq_c,
                "mask_in": mask_c,
                "w1m_in": w1m_host,
                "w1p_in": w1p_host,
                "packed_in": packed_host,
            }
        )
    return in_maps


def run(inputs, trace=False):
    """Run the kernel; returns ((main_logits, main_probs, fine), BassKernelResults)."""
    if "nc" not in _BASS_CACHE:
        _BASS_CACHE["nc"] = _build_module()
    nc = _BASS_CACHE["nc"]

    weights = {k: inputs[k] for k in inputs if k not in
               ("sequence_output", "entity_start_positions", "entity_end_positions")}
    in_maps = _prepare_inputs(
        inputs["sequence_output"],
        inputs["entity_start_positions"],
        inputs["entity_end_positions"],
        weights,
    )
    res = run_bass_kernel_spmd(nc, in_maps, core_ids=list(range(N_CORES)), trace=trace)
    allout = np.concatenate([r["out"] for r in res.results], axis=0)  # [512, 28]
    main_logits = np.ascontiguousarray(allout[:, 0:3])
    main_probs = np.ascontiguousarray(allout[:, 3:6])
    fine = np.ascontiguousarray(allout[:, 6:28])
    return (main_logits, main_probs, fine), res


def kernel(**inputs):
    outs, _ = run(inputs, trace=False)
    return outs


# revision 34
# speedup vs baseline: 1.0934x; 1.0042x over previous
"""Trainium2 Bass kernel for nn_EntityRoleClassifier (segment_reduce).

Strategy (data-parallel over batch, 8 NeuronCores):
  - Each core gets B/8 = 8 batch rows of sequence_output plus replicated MLP
    weights.
  - Per-entity left/span/right mean-pools are a dense matmul against
    host-built normalized interval masks (mask stationary, sequence moving),
    giving emb^T = mask^T @ seq in PSUM; DVE 32x32 block transposes land
    X^T (features on partitions) for the MLP.
  - The 4 head MLPs are fused: W1cat = [mW1|pW1|aW1|iW1] (2304x1024),
    W2blk = block-diagonal (1024x25).  The main head runs fp32 end-to-end
    (argmax near-ties need it); the p/a/i heads' W1 is bf16 (their sigmoids
    tolerate ~1e-4 error and it cuts weight DMA).
  - Softmax / argmax-select / sigmoid epilogue runs on-device, entity on
    partitions; output is one [64, 28] tile per core
    (3 main_logits | 3 main_probs | 22 fine).
"""

import numpy as np
import ml_dtypes

import concourse.bass as bass
import concourse.bacc as bacc
import concourse.mybir as mybir
import concourse.tile as tile
from concourse.bass_utils import run_bass_kernel_spmd

# Problem shapes (hardcoded; kernel.py must be self-contained).
B, S, H, E = 64, 512, 768, 8
HID = 256
N_CORES = 8
BPC = B // N_CORES          # batch rows per core = 8
NE = BPC * E                # entities per core = 64
TC = S // 128               # token chunks per row = 4
HC = H // 128               # h-chunks = 6
F3 = 3 * H                  # 2304
KC = F3 // 128              # feature chunks = 18
HID4 = 4 * HID              # 1024
HC2 = HID4 // 128           # 8
NOUT = 3 + 6 + 12 + 4       # 25
NCOLS = 3 + 3 + 22          # output tile columns = 28
MPAD = 32                   # mask columns: m = 4*j + seg, seg-3 lane zero
NPAI = HID4 - HID           # 768 p/a/i hidden units

F32 = mybir.dt.float32
BF16 = mybir.dt.bfloat16
DT = F32                    # sequence/mask dtype (fp32: argmax ties)

_BASS_CACHE = {}


def _build_module():
    nc = bacc.Bacc("TRN2", target_bir_lowering=False)
    seq_in = nc.dram_tensor("seq_in", [BPC, 128, TC * H], DT, kind="ExternalInput")
    mask_in = nc.dram_tensor("mask_in", [128, BPC * TC, MPAD], DT, kind="ExternalInput")
    w1m_in = nc.dram_tensor("w1m_in", [128, KC, HID], F32, kind="ExternalInput")
    w1p_in = nc.dram_tensor("w1p_in", [128, KC, NPAI], BF16, kind="ExternalInput")
    # packed smalls: [:, 0:8] b1 (hid-chunk layout), [:, 8:208] W2blk,
    # [0:64, 208:233] b2 broadcast
    packed_in = nc.dram_tensor("packed_in", [128, HC2 + HC2 * NOUT + NOUT], F32,
                               kind="ExternalInput")
    out = nc.dram_tensor("out", [NE, NCOLS], F32, kind="ExternalOutput")

    with tile.TileContext(nc) as tc:
        _body(tc, seq_in[:], mask_in[:], w1m_in[:], w1p_in[:], packed_in[:], out[:])
    if not nc.is_finalized():
        nc.finalize()
    return nc


def _body(tc, seq_ap, mask_ap, w1m_ap, w1p_ap, packed_ap, out_ap):
    nc = tc.nc
    X = mybir.AxisListType.X
    HH = H // 2          # 384 = one fp32 psum bank
    HCH = HC // 2        # 3 h-chunks per half
    with (
        tc.tile_pool(name="singles", bufs=1) as singles,
        tc.tile_pool(name="seqrow", bufs=1) as seqrow,
        tc.tile_pool(name="epi", bufs=1) as epi,
        tc.tile_pool(name="psum1", bufs=3, space="PSUM") as psum1,
        tc.tile_pool(name="psum2", bufs=1, space="PSUM") as psum2,
        tc.tile_pool(name="psum3", bufs=1, space="PSUM") as psum3,
    ):
        # --- all input DMAs, in lane-friendly order: 8 HWDGE semaphore lanes
        # round-robin in emission order, so DMAs 9..11 reuse lanes whose first
        # user finished long before (mask / row 0 / row 1).  Rows 0-3 are
        # single DMAs (prompt availability), rows 4-7 paired; smalls packed.
        seq_tiles = [None] * BPC
        seq_tiles[0] = seqrow.tile([128, TC, H], DT, name="seq0")  # lane 0, SP
        nc.sync.dma_start(seq_tiles[0], seq_ap[0].rearrange("p (t h) -> p t h", t=TC))
        mask_sb = singles.tile([128, BPC * TC, MPAD], DT)
        nc.scalar.dma_start(mask_sb, mask_ap)                     # lane 1, ACT
        for b in (1, 2, 3):                                       # lanes 2-4
            t_ = seqrow.tile([128, TC, H], DT, name=f"seq{b}")
            eng = nc.scalar if b % 2 == 1 else nc.sync
            eng.dma_start(t_, seq_ap[b].rearrange("p (t h) -> p t h", t=TC))
            seq_tiles[b] = t_
        pair45 = seqrow.tile([128, 2, TC, H], DT, name="seq45")   # lane 5, SP
        nc.sync.dma_start(pair45, seq_ap[4:6].rearrange("b p (t h) -> p b t h", t=TC))
        pair67 = seqrow.tile([128, 2, TC, H], DT, name="seq67")   # lane 6, ACT
        nc.scalar.dma_start(pair67, seq_ap[6:8].rearrange("b p (t h) -> p b t h", t=TC))
        for k in range(2):
            seq_tiles[4 + k] = pair45[:, k]
            seq_tiles[6 + k] = pair67[:, k]
        w1m_sb = singles.tile([128, KC, HID], F32)
        nc.sync.dma_start(w1m_sb, w1m_ap)                         # lane 7, SP
        w1p_sb = singles.tile([128, KC, NPAI], BF16)
        nc.scalar.dma_start(w1p_sb, w1p_ap)                       # lane 0 reuse
        packed_sb = singles.tile([128, HC2 + HC2 * NOUT + NOUT], F32)
        nc.scalar.dma_start(packed_sb, packed_ap)                 # lane 1 reuse
        b1_sb = packed_sb[:, 0:HC2]
        w2_sb = packed_sb[:, HC2 : HC2 + HC2 * NOUT].rearrange(
            "p (c n) -> p c n", n=NOUT
        )
        b2_sb = packed_sb[0:NE, HC2 + HC2 * NOUT :]

        # X^T: [feat-in-chunk, hchunk, row, mask] -> stage-2 lhsT slices
        xt_sb = singles.tile([128, HC, BPC, MPAD], F32)
        xtb_sb = singles.tile([128, HC, BPC, MPAD], BF16)

        # --- PE warm-up bridging until row 0 lands (HAM 4/8 -> 8/8), plus
        # ACT-table preloads so Exp/Sigmoid tables aren't loaded mid-epilogue
        warm_sb = singles.tile([128, HH], F32)
        nc.vector.memset(warm_sb, 0.0)
        warm_ps = psum1.tile([MPAD, HH], F32, tag="ps")
        for _ in range(8):
            nc.tensor.matmul(warm_ps, warm_sb[:, 0:MPAD], warm_sb,
                             start=True, stop=True)
        nc.vector.tensor_copy(warm_sb[0:MPAD, 0:1], warm_ps[:, 0:1])
        scrap_sb = singles.tile([1, 2], F32)
        nc.scalar.activation(scrap_sb[:, 0:1], warm_sb[0:1, 0:1],
                             mybir.ActivationFunctionType.Exp)
        nc.scalar.activation(scrap_sb[:, 1:2], warm_sb[0:1, 0:1],
                             mybir.ActivationFunctionType.Sigmoid)

        # --- stage 1: emb^T = mask^T @ seq (mask stationary) ----------------
        for b in range(BPC):
            seq_sb = seq_tiles[b]
            for half in range(2):
                ps = psum1.tile([MPAD, HH], F32)
                for t in range(TC):
                    nc.tensor.matmul(
                        ps,
                        mask_sb[:, b * TC + t, :],
                        seq_sb[:, t, half * HH : (half + 1) * HH],
                        start=(t == 0),
                        stop=(t == TC - 1),
                    )
                # ps[m, h'] with h' = hcw*128 + q*32 + p'; 32x32 block
                # transposes to xt[q*32+p', half*HCH+hcw, b, m]
                ps4 = ps.rearrange("m (hcw q p) -> m hcw q p", q=4, p=32)
                for q in range(4):
                    nc.vector.transpose(
                        xt_sb[q * 32 : (q + 1) * 32, half * HCH : (half + 1) * HCH, b, :],
                        ps4[:, :, q, :],
                    )
                # incremental bf16 cast for the p/a/i matmuls (ACT is idle)
                nc.scalar.copy(
                    xtb_sb[:, half * HCH : (half + 1) * HCH, b, :],
                    xt_sb[:, half * HCH : (half + 1) * HCH, b, :],
                )

        # --- stage 2: h = relu(X @ W1cat + b1), entities on partitions ------
        # lhsT slice over (b, j) for fixed (seg, hc): strides (32, 4) -> flat
        xtv = xt_sb.rearrange("p hc b (j s) -> p hc b j s", s=4)
        xtbv = xtb_sb.rearrange("p hc b (j s) -> p hc b j s", s=4)
        ps_m = psum2.tile([NE, HID], F32, name="ps_m")
        ps_pa = psum2.tile([NE, 512], F32, name="ps_pa")
        ps_pb = psum2.tile([NE, 128], F32, name="ps_pb")
        ps_pc = psum2.tile([NE, 128], F32, name="ps_pc")

        # h-pre transposed to [hid-in-chunk, c2, entity]; relu+bias after
        htr_sb = singles.tile([128, HC2, NE], F32)

        def transpose_hpre(ps, c2_base, n_c2):
            # ps[n, u'] with u' = c2'*128 + q*32 + p' -> htr[q*32+p', c2_base+c2', n]
            for r in range(2):
                p4 = ps[r * 32 : (r + 1) * 32, :].rearrange(
                    "n (c2 q p) -> n c2 q p", q=4, p=32
                )
                for q in range(4):
                    nc.vector.transpose(
                        htr_sb[q * 32 : (q + 1) * 32, c2_base : c2_base + n_c2,
                               r * 32 : (r + 1) * 32],
                        p4[:, :, q, :],
                    )

        for c in range(KC):
            seg, hc = divmod(c, HC)
            st, sp = (c == 0), (c == KC - 1)
            nc.tensor.matmul(ps_m, xtv[:, hc, :, :, seg], w1m_sb[:, c, :],
                             start=st, stop=sp)
        transpose_hpre(ps_m, 0, 2)

        # relu(h + b1) per 128-hid chunk, bias per-partition, into h^T layout.
        # On DVE (one fused op) so stage-3's matmuls aren't queued behind ACT.
        ht_sb = singles.tile([128, HC2, NE], F32)

        def relu_chunks(c2_base, n_c2):
            for c2 in range(c2_base, c2_base + n_c2):
                nc.vector.tensor_scalar(
                    ht_sb[:, c2, :], htr_sb[:, c2, :],
                    b1_sb[:, c2 : c2 + 1], 0.0,
                    mybir.AluOpType.add, mybir.AluOpType.max,
                )

        relu_chunks(0, 2)
        for c in range(KC):
            seg, hc = divmod(c, HC)
            st, sp = (c == 0), (c == KC - 1)
            nc.tensor.matmul(ps_pa, xtbv[:, hc, :, :, seg], w1p_sb[:, c, 0:512],
                             start=st, stop=sp)
        transpose_hpre(ps_pa, 2, 4)
        relu_chunks(2, 4)
        for c in range(KC):
            seg, hc = divmod(c, HC)
            st, sp = (c == 0), (c == KC - 1)
            nc.tensor.matmul(ps_pb, xtbv[:, hc, :, :, seg], w1p_sb[:, c, 512:640],
                             start=st, stop=sp)
        transpose_hpre(ps_pb, 6, 1)
        relu_chunks(6, 1)
        for c in range(KC):
            seg, hc = divmod(c, HC)
            st, sp = (c == 0), (c == KC - 1)
            nc.tensor.matmul(ps_pc, xtbv[:, hc, :, :, seg], w1p_sb[:, c, 640:NPAI],
                             start=st, stop=sp)
        transpose_hpre(ps_pc, 7, 1)
        relu_chunks(7, 1)

        # --- stage 3: logits = h @ W2blk + b2, entity on partitions ---------
        # (c2 6,7 last: their h^T chunks come off the final pai psum, so the
        # earlier chunks' matmuls overlap those transposes)
        ps3 = psum3.tile([NE, NOUT], F32)
        for i, c2 in enumerate([0, 1, 2, 3, 4, 5, 6, 7]):
            nc.tensor.matmul(
                ps3,
                ht_sb[:, c2, :],
                w2_sb[:, c2, :],
                start=(i == 0),
                stop=(i == HC2 - 1),
            )
        logits = epi.tile([NE, NOUT], F32)
        nc.vector.tensor_add(logits, ps3, b2_sb)

        # --- epilogue --------------------------------------------------------
        outsb = epi.tile([NE, NCOLS], F32)
        nc.vector.tensor_copy(outsb[:, 0:3], logits[:, 0:3])

        # softmax over the 3 main logits
        rmax = epi.tile([NE, 1], F32)
        nc.vector.reduce_max(rmax, logits[:, 0:3], axis=X)
        negmax = epi.tile([NE, 1], F32)
        nc.vector.tensor_scalar_mul(negmax, rmax, -1.0)
        exps = epi.tile([NE, 3], F32)
        ssum = epi.tile([NE, 1], F32)
        nc.scalar.activation(
            exps, logits[:, 0:3], mybir.ActivationFunctionType.Exp,
            bias=negmax, scale=1.0, accum_out=ssum,
        )
        rinv = epi.tile([NE, 1], F32)
        nc.vector.reciprocal(rinv, ssum)
        nc.vector.tensor_scalar_mul(outsb[:, 3:6], exps, rinv)

        # argmax-select masks (first-max-wins, matching jnp.argmax)
        L0, L1, L2 = logits[:, 0:1], logits[:, 1:2], logits[:, 2:3]
        ge01 = epi.tile([NE, 1], F32)
        nc.vector.tensor_tensor(ge01, L0, L1, mybir.AluOpType.is_ge)
        ge02 = epi.tile([NE, 1], F32)
        nc.vector.tensor_tensor(ge02, L0, L2, mybir.AluOpType.is_ge)
        ge12 = epi.tile([NE, 1], F32)
        nc.vector.tensor_tensor(ge12, L1, L2, mybir.AluOpType.is_ge)
        is0 = epi.tile([NE, 1], F32)
        nc.vector.tensor_mul(is0, ge01, ge02)
        not01 = epi.tile([NE, 1], F32)
        nc.vector.tensor_scalar(
            not01, ge01, -1.0, 1.0, mybir.AluOpType.mult, mybir.AluOpType.add
        )
        is1 = epi.tile([NE, 1], F32)
        nc.vector.tensor_mul(is1, not01, ge12)
        is01 = epi.tile([NE, 1], F32)
        nc.vector.tensor_add(is01, is0, is1)
        is2 = epi.tile([NE, 1], F32)
        nc.vector.tensor_scalar(
            is2, is01, -1.0, 1.0, mybir.AluOpType.mult, mybir.AluOpType.add
        )

        sig = epi.tile([NE, 22], F32)
        nc.scalar.activation(sig, logits[:, 3:NOUT], mybir.ActivationFunctionType.Sigmoid)
        nc.vector.tensor_scalar_mul(outsb[:, 6:12], sig[:, 0:6], is0)
        nc.vector.tensor_scalar_mul(outsb[:, 12:24], sig[:, 6:18], is1)
        nc.vector.tensor_scalar_mul(outsb[:, 24:28], sig[:, 18:22], is2)

        nc.sync.dma_start(out_ap, outsb)


def _prepare_inputs(sequence_output, starts, ends, weights):
    """Host-side packing: masks, fused weights, per-core shards."""
    seq = np.ascontiguousarray(np.asarray(sequence_output, dtype=np.float32))
    s = np.asarray(starts).astype(np.int64)
    e = np.asarray(ends).astype(np.int64)

    t = np.arange(S)
    left_w = np.where(s > 0, 1.0 / np.maximum(s, 1), 0.0)          # [B,E]
    span_w = 1.0 / (e - s + 1)
    right_cnt = S - (e + 1)
    right_w = np.where(right_cnt > 0, 1.0 / np.maximum(right_cnt, 1), 0.0)

    lm = (t[None, None, :] < s[:, :, None]) * left_w[:, :, None]    # [B,E,S]
    sm = ((t[None, None, :] >= s[:, :, None]) & (t[None, None, :] <= e[:, :, None])) * span_w[:, :, None]
    rm = (t[None, None, :] > e[:, :, None]) * right_w[:, :, None]
    mask_bes = np.stack([lm, sm, rm], axis=1)                       # [B,3,E,S]
    # mask column order m = 4*j + seg (seg=3 lane zero-padded) so the stage-2
    # stationary slice over (b, j) flattens to a single strided free dim
    maskT4 = np.zeros((B, S, E, 4), np.float32)
    maskT4[:, :, :, :3] = mask_bes.transpose(0, 3, 2, 1)            # [B,S,E,3]
    maskT = maskT4.reshape(B, S, MPAD)

    mW1, pW1, aW1, iW1 = (np.asarray(weights[k], np.float32) for k in ("mW1", "pW1", "aW1", "iW1"))
    mb1, pb1, ab1, ib1 = (np.asarray(weights[k], np.float32) for k in ("mb1", "pb1", "ab1", "ib1"))
    mW2, pW2, aW2, iW2 = (np.asarray(weights[k], np.float32) for k in ("mW2", "pW2", "aW2", "iW2"))
    mb2, pb2, ab2, ib2 = (np.asarray(weights[k], np.float32) for k in ("mb2", "pb2", "ab2", "ib2"))

    w1m_host = np.ascontiguousarray(
        mW1.reshape(KC, 128, HID).transpose(1, 0, 2)
    )                                                               # [128, KC, 256]
    W1pai = np.concatenate([pW1, aW1, iW1], axis=1)                 # [2304, 768]
    w1p_host = np.ascontiguousarray(
        W1pai.reshape(KC, 128, NPAI).transpose(1, 0, 2).astype(ml_dtypes.bfloat16)
    )                                                               # [128, KC, 768]
    b1cat = np.concatenate([mb1, pb1, ab1, ib1]).astype(np.float32)  # [1024]
    b1_host = b1cat.reshape(HC2, 128).T                              # [128, 8]

    W2blk = np.zeros((HID4, NOUT), np.float32)
    W2blk[0:256, 0:3] = mW2
    W2blk[256:512, 3:9] = pW2
    W2blk[512:768, 9:21] = aW2
    W2blk[768:1024, 21:25] = iW2
    w2_host = W2blk.reshape(HC2, 128, NOUT).transpose(1, 0, 2)       # [128, 8, 25]
    b2cat = np.concatenate([mb2, pb2, ab2, ib2]).astype(np.float32)  # [25]

    packed_host = np.zeros((128, HC2 + HC2 * NOUT + NOUT), np.float32)
    packed_host[:, 0:HC2] = b1_host
    packed_host[:, HC2 : HC2 + HC2 * NOUT] = w2_host.reshape(128, HC2 * NOUT)
    packed_host[0:NE, HC2 + HC2 * NOUT :] = np.broadcast_to(b2cat, (NE, NOUT))

    in_maps = []
    for c in range(N_CORES):
        bs = slice(c * BPC, (c + 1) * BPC)
        # [BPC, 128, TC*H]: contiguous 12KB per-partition lines
        seq_c = np.ascontiguousarray(
            seq[bs].reshape(BPC, TC, 128, H).transpose(0, 2, 1, 3).reshape(BPC, 128, TC * H)
        )
        mask_c = np.ascontiguousarray(
            maskT[bs].reshape(BPC, TC, 128, MPAD).transpose(2, 0, 1, 3)
            .reshape(128, BPC * TC, MPAD)
        )
        in_maps.append(
            {
                "seq_in": seq_c,
                "mask_in": mask_c,
                "w1m_in": w1m_host,
                "w1p_in": w1p_host,
                "packed_in": packed_host,
            }
        )
    return in_maps


def run(inputs, trace=False):
    """Run the kernel; returns ((main_logits, main_probs, fine), BassKernelResults)."""
    if "nc" not in _BASS_CACHE:
        _BASS_CACHE["nc"] = _build_module()
    nc = _BASS_CACHE["nc"]

    weights = {k: inputs[k] for k in inputs if k not in
               ("sequence_output", "entity_start_positions", "entity_end_positions")}
    in_maps = _prepare_inputs(
        inputs["sequence_output"],
        inputs["entity_start_positions"],
        inputs["entity_end_positions"],
        weights,
    )
    res = run_bass_kernel_spmd(nc, in_maps, core_ids=list(range(N_CORES)), trace=trace)
    allout = np.concatenate([r["out"] for r in res.results], axis=0)  # [512, 28]
    main_logits = np.ascontiguousarray(allout[:, 0:3])
    main_probs = np.ascontiguousarray(allout[:, 3:6])
    fine = np.ascontiguousarray(allout[:, 6:28])
    return (main_logits, main_probs, fine), res


def kernel(**inputs):
    outs, _ = run(inputs, trace=False)
    return outs
